# revision 2
# baseline (speedup 1.0000x reference)
"""CrossAndCompress Trainium2 kernel.

Reference computation (per row r of the batch):
    a_r = enc_item[r] . theta_vv        b_r = enc_user[r] . theta_ev
    c_r = enc_item[r] . theta_ve        d_r = enc_user[r] . theta_ee
    v_out[r] = enc_user[r] * a_r + enc_item[r] * b_r + beta_v
    e_out[r] = enc_user[r] * c_r + enc_item[r] * d_r + beta_e

Sharding: pure data parallel — batch dim (16384) split across 8 NeuronCores
(2048 rows each); the tiny theta/beta vectors are replicated (host-side
pre-broadcast to 128 partitions so DVE can consume them directly).

Per-core pipeline over 16 tiles of [128 rows x 1024]:
  - DMA in user/item tiles (natural layout: rows on partitions, contiguous HBM)
  - 4 fused multiply+reduce dots on VectorE (tensor_tensor_reduce)
  - per-row scaled products split between ScalarE (activation, per-partition
    scale AP) and VectorE (tensor_scalar_mul)
  - the two adds run on TensorE as identity-matmul PSUM accumulation
  - ScalarE evacuates PSUM -> SBUF, DMA out
"""

import numpy as np

B, D = 16384, 1024
N_CORES = 8
ROWS_PER_CORE = B // N_CORES  # 2048
TILE_P = 128
N_TILES = ROWS_PER_CORE // TILE_P  # 16

_PROGRAM_CACHE: dict = {}


def _build_program(with_beta: bool):
    import concourse.bass as bass  # noqa: F401
    import concourse.mybir as mybir
    import concourse.tile as tile
    from concourse import bacc

    f32 = mybir.dt.float32
    AF = mybir.ActivationFunctionType
    OP = mybir.AluOpType

    nc = bacc.Bacc(
        "TRN2",
        target_bir_lowering=False,
        debug=False,
        enable_asserts=False,
        num_devices=N_CORES,
    )

    u_h = nc.dram_tensor(
        "enc_user", [ROWS_PER_CORE, D], f32, kind="ExternalInput"
    ).ap()
    i_h = nc.dram_tensor(
        "enc_item", [ROWS_PER_CORE, D], f32, kind="ExternalInput"
    ).ap()
    th_h = nc.dram_tensor("thetas", [TILE_P, 4 * D], f32, kind="ExternalInput").ap()
    id_h = nc.dram_tensor("ident", [TILE_P, TILE_P], f32, kind="ExternalInput").ap()
    if with_beta:
        be_h = nc.dram_tensor("betas", [TILE_P, 2 * D], f32, kind="ExternalInput").ap()
    v_h = nc.dram_tensor("v_out", [ROWS_PER_CORE, D], f32, kind="ExternalOutput").ap()
    e_h = nc.dram_tensor("e_out", [ROWS_PER_CORE, D], f32, kind="ExternalOutput").ap()

    with tile.TileContext(nc) as tc:
        with (
            tc.tile_pool(name="const", bufs=1) as cpool,
            tc.tile_pool(name="io", bufs=3) as io,
            tc.tile_pool(name="work", bufs=3) as work,
            tc.tile_pool(name="ps", bufs=4, space="PSUM") as ps,
        ):
            th = cpool.tile([TILE_P, 4 * D], f32)
            nc.sync.dma_start(th[:], th_h[:, :])
            ident = cpool.tile([TILE_P, TILE_P], f32)
            nc.sync.dma_start(ident[:], id_h[:, :])
            if with_beta:
                betas = cpool.tile([TILE_P, 2 * D], f32)
                nc.sync.dma_start(betas[:], be_h[:, :])

            # theta layout along free dim: [t_vv | t_ev | t_ve | t_ee]
            t_sl = [th[:, k * D : (k + 1) * D] for k in range(4)]

            for i in range(N_TILES):
                rows = slice(i * TILE_P, (i + 1) * TILE_P)
                u = io.tile([TILE_P, D], f32, tag="u")
                it = io.tile([TILE_P, D], f32, tag="it")
                nc.sync.dma_start(u[:], u_h[rows, :])
                nc.sync.dma_start(it[:], i_h[rows, :])

                # dots[:, 0..3] = a, b, c, d
                dots = work.tile([TILE_P, 4], f32, tag="dots")
                for k, src in enumerate((it, u, it, u)):
                    scr = work.tile([TILE_P, D], f32, tag="scr")
                    nc.vector.affine_mul_reduce(
                        out=scr[:],
                        accum_out=dots[:, k : k + 1],
                        in0=src[:],
                        in1=t_sl[k],
                        scale=1.0,
                        bias=0.0,
                    )

                # scaled products: p1 = u*a (ACT), p2 = it*b (DVE),
                #                  p3 = u*c (ACT), p4 = it*d (ACT)
                p1 = work.tile([TILE_P, D], f32, tag="p1")
                nc.scalar.activation(p1[:], u[:], AF.Copy, bias=0.0, scale=dots[:, 0:1])
                p2 = work.tile([TILE_P, D], f32, tag="p2")
                nc.vector.tensor_scalar_mul(p2[:], it[:], dots[:, 1:2])
                p3 = work.tile([TILE_P, D], f32, tag="p3")
                nc.scalar.activation(p3[:], u[:], AF.Copy, bias=0.0, scale=dots[:, 2:3])
                p4 = work.tile([TILE_P, D], f32, tag="p4")
                nc.scalar.activation(p4[:], it[:], AF.Copy, bias=0.0, scale=dots[:, 3:4])

                # adds on TensorE: v = I @ p1 + I @ p2, e = I @ p3 + I @ p4
                vps = ps.tile([TILE_P, D], f32, tag="ps")
                eps = ps.tile([TILE_P, D], f32, tag="ps")
                for h in range(2):
                    sl = slice(h * 512, (h + 1) * 512)
                    nc.tensor.matmul(vps[:, sl], ident[:], p1[:, sl], start=True, stop=False)
                    nc.tensor.matmul(vps[:, sl], ident[:], p2[:, sl], start=False, stop=True)
                    nc.tensor.matmul(eps[:, sl], ident[:], p3[:, sl], start=True, stop=False)
                    nc.tensor.matmul(eps[:, sl], ident[:], p4[:, sl], start=False, stop=True)

                v_sb = io.tile([TILE_P, D], f32, tag="v_sb")
                e_sb = io.tile([TILE_P, D], f32, tag="e_sb")
                nc.scalar.activation(v_sb[:], vps[:], AF.Copy)
                nc.scalar.activation(e_sb[:], eps[:], AF.Copy)
                if with_beta:
                    v_sb2 = io.tile([TILE_P, D], f32, tag="v_sb2")
                    e_sb2 = io.tile([TILE_P, D], f32, tag="e_sb2")
                    nc.vector.tensor_add(v_sb2[:], v_sb[:], betas[:, 0:D])
                    nc.vector.tensor_add(e_sb2[:], e_sb[:], betas[:, D : 2 * D])
                    v_sb, e_sb = v_sb2, e_sb2
                nc.sync.dma_start(v_h[rows, :], v_sb[:])
                nc.sync.dma_start(e_h[rows, :], e_sb[:])

    nc.compile()
    return nc


def _get_program(with_beta: bool):
    if with_beta not in _PROGRAM_CACHE:
        _PROGRAM_CACHE[with_beta] = _build_program(with_beta)
    return _PROGRAM_CACHE[with_beta]


def _prep_host_inputs(inputs):
    enc_user = np.ascontiguousarray(np.asarray(inputs["enc_user"], dtype=np.float32))
    enc_item = np.ascontiguousarray(np.asarray(inputs["enc_item"], dtype=np.float32))
    assert enc_user.shape == (B, D) and enc_item.shape == (B, D)

    def vec(name):
        return np.asarray(inputs[name], dtype=np.float32).reshape(D)

    thetas = np.concatenate(
        [vec("theta_vv"), vec("theta_ev"), vec("theta_ve"), vec("theta_ee")]
    )
    thetas_b = np.ascontiguousarray(
        np.broadcast_to(thetas[None, :], (TILE_P, 4 * D))
    )
    beta_v, beta_e = vec("beta_v"), vec("beta_e")
    with_beta = bool(np.any(beta_v) or np.any(beta_e))
    betas_b = None
    if with_beta:
        betas_b = np.ascontiguousarray(
            np.broadcast_to(
                np.concatenate([beta_v, beta_e])[None, :], (TILE_P, 2 * D)
            )
        )
    ident = np.eye(TILE_P, dtype=np.float32)
    return enc_user, enc_item, thetas_b, betas_b, ident, with_beta


def _make_in_maps(enc_user, enc_item, thetas_b, betas_b, ident, with_beta):
    in_maps = []
    for c in range(N_CORES):
        rows = slice(c * ROWS_PER_CORE, (c + 1) * ROWS_PER_CORE)
        m = {
            "enc_user": np.ascontiguousarray(enc_user[rows]),
            "enc_item": np.ascontiguousarray(enc_item[rows]),
            "thetas": thetas_b,
            "ident": ident,
        }
        if with_beta:
            m["betas"] = betas_b
        in_maps.append(m)
    return in_maps


def run_on_hw(inputs, trace=False):
    """Build/fetch the program, run it SPMD on 8 cores, gather outputs.

    Returns ((v_out, e_out), BassKernelResults).
    """
    from concourse.bass_utils import run_bass_kernel_spmd

    host = _prep_host_inputs(inputs)
    with_beta = host[-1]
    nc = _get_program(with_beta)
    in_maps = _make_in_maps(*host)
    res = run_bass_kernel_spmd(nc, in_maps, list(range(N_CORES)), trace=trace)
    v = np.concatenate([np.asarray(res.results[c]["v_out"]) for c in range(N_CORES)], axis=0)
    e = np.concatenate([np.asarray(res.results[c]["e_out"]) for c in range(N_CORES)], axis=0)
    return (v, e), res


def kernel(**inputs):
    (v, e), _ = run_on_hw(inputs, trace=False)
    return v, e
